# revision 13
# baseline (speedup 1.0000x reference)
"""CTC loss (keras ctc_batch_cost semantics) as a Bass/Tile kernel on 8
TRN2 NeuronCores.

Strategy (per core, 64 examples):
  - Linear-space CTC forward DP as a wavefront over the 65 extended
    states; each state's full time series is ONE DVE tensor_tensor_scan
    (state = (inflow[t-1] + state) * p[t]).
  - Time is split fwd/bwd: partition rows 0..63 run the forward DP over
    t in [0,256) and rows 64..127 run the backward DP over t in [256,512)
    (s- and t-reversed). Host combines the two halves per example.
  - Gather of p[s,t] = K*y_pred[e,t,ext[s]]: per example ONE 256KB DMA
    loads y_pred[e] as [t%128, (tt,c)]; 4 PE transposes -> [c,t] PSUM;
    DVE copy casts to bf16 SBUF; 2 PE matmuls against host-built bf16
    one-hots (fwd + s-reversed bwd) -> [65, 512] PSUM; ACT copies move
    fwd (as-is) and bwd (time-REVERSED via negative-stride read, which
    is free on compute engines - never reverse in a DMA: that degenerates
    to per-element descriptors and was the old 6ms bottleneck) into a
    bf16 staging buffer gall[s, (e,t)].
  - Every 16 examples, one flatten DMA per direction redistributes
    gall -> pstore[(e,d), (s,t)] (512B-run descriptors, fast).
  - Scaling: K = 94.5 (exact in bf16) per step keeps the fp32 DP in
    range for 256 steps; host removes T*log(K) at the end.
"""
import contextlib
import ctypes
import sys
import types

import numpy as np

sys.path.insert(0, "/opt/trn_rl_repo")

B, T, C, L = 512, 512, 128, 32
BLANK = C - 1
S = 2 * L + 1            # 65 extended states
TH = T // 2              # 256 timesteps per direction
NCORES = 8
EX_PER_CORE = B // NCORES  # 64
K_SCALE = 94.5           # exactly representable in bf16
KLOG = float(np.log(K_SCALE))
BLK = TH + 1             # alpha-store block stride (guard col + 256)


# ---------------------------------------------------------------------------
# axon runtime shims (NTFF profile hook + no-op artifact upload)
# ---------------------------------------------------------------------------
_SO_PATH = "/opt/axon/libaxon_pjrt.so"


def _make_ntff_hook():
    try:
        lib = ctypes.CDLL(_SO_PATH)
    except OSError:
        return None
    if not hasattr(lib, "axon_start_nrt_profile"):
        return None
    lib.axon_start_nrt_profile.argtypes = [
        ctypes.POINTER(ctypes.c_int64),
        ctypes.c_size_t,
    ]
    lib.axon_start_nrt_profile.restype = ctypes.c_int64
    lib.axon_stop_nrt_profile.argtypes = [ctypes.c_char_p]
    lib.axon_stop_nrt_profile.restype = ctypes.c_int64

    @contextlib.contextmanager
    def _hook(output_dir, device_ids):
        import jax

        jax.devices()
        if device_ids:
            ids = (ctypes.c_int64 * len(device_ids))(*device_ids)
            rc = lib.axon_start_nrt_profile(ids, len(device_ids))
        else:
            rc = lib.axon_start_nrt_profile(None, 0)
        if rc != 0:
            raise RuntimeError(f"axon_start_nrt_profile rc={rc}")
        try:
            yield
        finally:
            lib.axon_stop_nrt_profile(str(output_dir).encode())

    return _hook


def _install_shims():
    if "antenv.axon_hooks" not in sys.modules:
        mod = types.ModuleType("antenv.axon_hooks")
        hook = _make_ntff_hook()
        mod.get_axon_ntff_profile_hook = lambda: hook
        mod.set_axon_ntff_profile_hook = lambda h: None
        sys.modules["antenv.axon_hooks"] = mod
    import concourse.bass_utils as bu

    bu.upload_artifacts = lambda tmpdir: str(tmpdir)


# ---------------------------------------------------------------------------
# device program
# ---------------------------------------------------------------------------
_NC_CACHE = {}


def build_program():
    _install_shims()
    import concourse.bacc as bacc
    import concourse.mybir as mybir
    from concourse.masks import make_identity
    from concourse.tile import TileContext

    F32 = mybir.dt.float32
    BF16 = mybir.dt.bfloat16
    ALU = mybir.AluOpType

    nc = bacc.Bacc("TRN2")
    yp = nc.dram_tensor("yp", [EX_PER_CORE, T, C], F32, kind="ExternalInput")
    oh = nc.dram_tensor("oh", [C, EX_PER_CORE * 2 * S], BF16,
                        kind="ExternalInput")
    msk = nc.dram_tensor("msk", [128, S], F32, kind="ExternalInput")
    w_out = nc.dram_tensor("W", [128, S], F32, kind="ExternalOutput")

    with TileContext(nc) as tc:
        with (
            tc.tile_pool(name="persist", bufs=1) as persist,
            tc.tile_pool(name="slabp", bufs=3) as slabp,
            tc.tile_pool(name="slabTp", bufs=3) as slabTp,
            tc.tile_pool(name="gsbp", bufs=4) as gsbp,
            tc.tile_pool(name="upool", bufs=2) as upool,
            tc.tile_pool(name="psT", bufs=3, space="PSUM") as psT,
            tc.tile_pool(name="psG", bufs=3, space="PSUM") as psG,
        ):
            ident = persist.tile([128, 128], F32, tag="ident")
            ohs = persist.tile([C, EX_PER_CORE * 2 * S], BF16, tag="ohs")
            msk_sb = persist.tile([128, S], F32, tag="msk")
            pstore = persist.tile([128, S * TH], BF16, tag="pstore")
            astore = persist.tile([128, (S + 2) * BLK], F32, tag="astore")
            bnd = persist.tile([128, S], F32, tag="bnd")

            nc.scalar.dma_start(ohs[:, :], oh[:, :])
            nc.scalar.dma_start(msk_sb[:, :], msk[:, :])
            make_identity(nc, ident[:, :])

            # alpha store init: zeros everywhere; backward rows get guard
            # value 1.0 on iteration blocks 0 and 1 (end states 64, 63).
            nc.gpsimd.memset(astore[:, :], 0.0)
            nc.vector.memset(astore[64:128, 2 * BLK : 2 * BLK + 1], 1.0)
            nc.vector.memset(astore[64:128, 3 * BLK : 3 * BLK + 1], 1.0)

            # ---------------- gather phase ----------------
            # Load layout: partition p = t-quad (t = 4p + j), so each
            # descriptor is one 2KB contiguous run y_pred[e, 4p:4p+4, :].
            # Columns of the transposed slab come out quad-permuted
            # (col (j,p) <-> t=4p+j); the matmul rhs APs select the
            # fwd/bwd halves (p<64 / p>=64) and the ACT copies unpermute.
            # Software-pipelined: example e's matmuls/copies/scatters are
            # emitted one iteration behind its transposes+cast, so PE's
            # strict-FIFO matmul queue never stalls on the DVE cast.
            sTs = {}
            for e in range(EX_PER_CORE + 1):
                if e < EX_PER_CORE:
                    if e % 8 == 0:
                        slab8 = slabp.tile([128, 8 * T], F32, tag="slab8")
                        q = nc.sync if (e // 8) % 2 == 0 else nc.scalar
                        q.dma_start(
                            slab8[:, :].rearrange(
                                "p (el j c) -> p el j c", c=C, j=4
                            ),
                            yp[e : e + 8, :, :].rearrange(
                                "el (p j) c -> p el j c", j=4
                            ),
                        )
                    el0 = (e % 8) * T
                    sT_ps = psT.tile([128, T], F32, tag="sT_ps")
                    for j in range(4):
                        nc.tensor.transpose(
                            sT_ps[:, j * 128 : (j + 1) * 128],
                            slab8[:, el0 + j * 128 : el0 + (j + 1) * 128],
                            ident[:, :],
                        )
                    sT = slabTp.tile([128, T], BF16, tag="sT")
                    nc.vector.tensor_copy(sT[:, :], sT_ps[:, :])
                    sTs[e] = sT
                if e < 1:
                    continue
                ee = e - 1
                sTv = sTs.pop(ee)[:, :].rearrange("k (j p) -> k j p", p=128)
                g_ps = psG.tile([S, T], F32, tag="g_ps")
                nc.tensor.matmul(
                    g_ps[:, 0:TH],
                    ohs[:, (2 * ee) * S : (2 * ee + 1) * S],
                    sTv[:, :, 0:64],
                    start=True, stop=True,
                )
                nc.tensor.matmul(
                    g_ps[:, TH:T],
                    ohs[:, (2 * ee + 1) * S : (2 * ee + 2) * S],
                    sTv[:, :, 64:128],
                    start=True, stop=True,
                )
                # unpermute (j,p) -> t = 4p+j; bwd additionally
                # time-reversed (negative strides, compute engine).
                gsb = gsbp.tile([S, T], BF16, tag="gsb")
                nc.scalar.copy(
                    gsb[:, 0:TH].rearrange("s (p j) -> s j p", j=4),
                    g_ps[:, 0:TH].rearrange("s (j p) -> s j p", p=64),
                )
                nc.scalar.copy(
                    gsb[:, TH:T].rearrange("s (p j) -> s p j", j=4)[
                        :, ::-1, ::-1
                    ],
                    g_ps[:, TH:T].rearrange("s (j p) -> s p j", p=64),
                )
                # per-(e,d) scatter: dst ONE partition row, src
                # partition-major [65, 256] (proven-fast pattern).
                for d in range(2):
                    row = d * 64 + ee
                    q2 = nc.sync if (ee + d) % 2 == 0 else nc.scalar
                    q2.dma_start(
                        pstore[row : row + 1, :].rearrange(
                            "a (s t) -> a s t", t=TH
                        ),
                        gsb[:, d * TH : (d + 1) * TH],
                    )

            # ---------------- wavefront ----------------
            for i in range(S):
                u = upool.tile([128, BLK], F32, tag="u")
                nc.vector.scalar_tensor_tensor(
                    u[:, :],
                    astore[:, i * BLK : i * BLK + BLK],
                    msk_sb[:, i : i + 1],
                    astore[:, (i + 1) * BLK : (i + 1) * BLK + BLK],
                    ALU.mult,
                    ALU.add,
                )
                ob = (i + 2) * BLK
                nc.vector.tensor_tensor_scan(
                    astore[:, ob + 1 : ob + 1 + TH],
                    u[:, 0:TH],
                    pstore[:, i * TH : (i + 1) * TH],
                    1.0 if i < 2 else 0.0,
                    ALU.add,
                    ALU.mult,
                )

            # boundary column t = TH-1 of every state: pack via DVE
            # (strided read), then ONE dense DMA out.
            bndv = astore[:, :].rearrange("p (s c) -> p s c", c=BLK)[
                :, 2 : 2 + S, TH : TH + 1
            ]
            nc.vector.tensor_copy(
                bnd[:, :], bndv.rearrange("p s c -> p (s c)")
            )
            nc.sync.dma_start(w_out[:, :], bnd[:, :])

    nc.finalize()
    return nc


def _get_program():
    if "nc" not in _NC_CACHE:
        _NC_CACHE["nc"] = build_program()
    return _NC_CACHE["nc"]


# ---------------------------------------------------------------------------
# host side
# ---------------------------------------------------------------------------
def _host_prep(y_true, y_pred):
    import ml_dtypes

    bf16 = ml_dtypes.bfloat16
    y_true = np.asarray(y_true)
    y_pred = np.asarray(y_pred, dtype=np.float32)
    ext = np.full((B, S), BLANK, np.int64)
    ext[:, 1::2] = y_true.astype(np.int64)
    skip = np.zeros((B, S), bool)
    skip[:, 2:] = (ext[:, 2:] != BLANK) & (ext[:, 2:] != ext[:, :-2])
    K = np.float32(K_SCALE)

    in_maps = []
    for k in range(NCORES):
        sl = slice(k * EX_PER_CORE, (k + 1) * EX_PER_CORE)
        exk = ext[sl]                              # [64, S]
        # one-hot [c, (e, d, s)]: col e*130 + d*65 + s
        ohk = np.zeros((C, EX_PER_CORE, 2, S), np.float32)
        r_idx = np.arange(EX_PER_CORE)[None, :, None]
        s_idx = np.arange(S)[None, None, :]
        ohk[exk[None, :, :], r_idx, 0, s_idx] = K
        ohk[exk[:, ::-1][None, :, :], r_idx, 1, s_idx] = K
        mskk = np.zeros((128, S), np.float32)
        mskk[:EX_PER_CORE] = skip[sl].astype(np.float32)
        # backward rows: iteration i targets state 64-i; its skip inflow
        # comes from state 66-i (mask skip[66-i], zero when out of range).
        sk = np.zeros((EX_PER_CORE, S), np.float32)
        sk[:, : S - 2] = skip[sl, 2:].astype(np.float32)
        mskk[EX_PER_CORE:] = sk[:, ::-1]
        in_maps.append(
            {
                "yp": np.ascontiguousarray(y_pred[sl]),
                "oh": np.ascontiguousarray(
                    ohk.reshape(C, EX_PER_CORE * 2 * S)
                ).astype(bf16),
                "msk": mskk,
            }
        )
    return in_maps, ext, skip


def _host_combine(Ws, skip):
    loss = np.zeros((B, 1), np.float32)
    for k in range(NCORES):
        Wk = Ws[k].astype(np.float64)
        for r in range(EX_PER_CORE):
            e = k * EX_PER_CORE + r
            wf = Wk[r]                       # alpha[s, 255]
            wb = Wk[EX_PER_CORE + r][::-1]   # B[s, 256]
            a2 = wf.copy()
            a2[1:] += wf[:-1]
            a2[2:] += np.where(skip[e, 2:], wf[:-2], 0.0)
            ptot = float((a2 * wb).sum())
            loss[e, 0] = -(np.log(ptot) - T * KLOG)
    return loss


def kernel(y_true, y_pred, trace=False):
    _install_shims()
    from concourse.bass_utils import run_bass_kernel_spmd

    nc = _get_program()
    in_maps, ext, skip = _host_prep(y_true, y_pred)
    res = run_bass_kernel_spmd(
        nc, in_maps, list(range(NCORES)), trace=trace
    )
    Ws = [res.results[k]["W"] for k in range(NCORES)]
    loss = _host_combine(Ws, skip)
    if trace:
        kernel.last_exec_time_ns = res.exec_time_ns
    return loss
